# revision 8
# baseline (speedup 1.0000x reference)
"""Bass/Tile TRN2 kernel for nn_BasicTransformerBlock (B=2,N=2048,D=1024,H=16,DH=64,GEGLU).

Sharding: 8 cores, 2 heads/core for attention (head-parallel), token-sharded
Wo+FF (512 tokens/core) after an AllToAll exchange of the attention output.
All matmuls in float32r (full PE rate, ~1e-4 rounding).
"""
import os
import numpy as np
import concourse.bass as bass
import concourse.tile as tile
from concourse import bacc, mybir
from concourse import bass_utils
from concourse.masks import make_identity

F32 = mybir.dt.float32
F32R = mybir.dt.float32r
AF = mybir.ActivationFunctionType
ALU = mybir.AluOpType

N_CORES = 8
B, N, D, H, DH = 2, 2048, 1024, 16, 64
TOK = B * N                    # 4096
FFI = 4 * D                    # 4096
EPS = 1e-5
TPC = TOK // N_CORES           # 512 tokens per core (consumer side)
KD = D // 128                  # 8 k-tiles over D
PAIRS = [(b, hl) for b in range(B) for hl in range(2)]   # 4 (batch, local-head) pairs


def _ln_stats(nc, pool, src_ap, eps_t):
    """Row LayerNorm stats of src_ap [128, 1024] -> (rstd [128,1], nbias [128,1])."""
    st = pool.tile([128, 2, 6], F32, tag="lnst")
    nc.vector.bn_stats(out=st[:, 0, :], in_=src_ap[:, 0:512])
    nc.vector.bn_stats(out=st[:, 1, :], in_=src_ap[:, 512:1024])
    mv = pool.tile([128, 2], F32, tag="lnmv")
    nc.vector.bn_aggr(out=mv[:], in_=st[:])
    lnv = pool.tile([128, 1], F32, tag="lnv")
    nc.scalar.activation(lnv[:], mv[:, 1:2], AF.Ln, bias=eps_t[:])
    rstd = pool.tile([128, 1], F32, tag="lnrstd")
    nc.scalar.activation(rstd[:], lnv[:], AF.Exp, scale=-0.5)
    nbias = pool.tile([128, 1], F32, tag="lnnb")
    nc.vector.tensor_scalar(nbias[:], mv[:, 0:1], rstd[:], -1.0,
                            op0=ALU.mult, op1=ALU.mult)
    return rstd, nbias


def build_program(has_bq, has_bk, has_bv, has_bo, has_b1a, has_b1g, has_b2):
    nc = bacc.Bacc("TRN2", target_bir_lowering=False, debug=False,
                   num_devices=N_CORES)

    x_in = nc.dram_tensor("x", [TOK, D], F32, kind="ExternalInput").ap()
    xres_in = nc.dram_tensor("x_res", [TPC, D], F32, kind="ExternalInput").ap()
    wq_in = nc.dram_tensor("wq", [128, KD, 128], F32, kind="ExternalInput").ap()
    wk_in = nc.dram_tensor("wk", [128, KD, 128], F32, kind="ExternalInput").ap()
    wv_in = nc.dram_tensor("wv", [128, KD, 128], F32, kind="ExternalInput").ap()
    wo_in = nc.dram_tensor("wo", [128, KD, D], F32, kind="ExternalInput").ap()
    w1_in = nc.dram_tensor("wff1", [64, 128, KD, 128], F32, kind="ExternalInput").ap()
    w2_in = nc.dram_tensor("wff2", [32, 128, D], F32, kind="ExternalInput").ap()
    bqr_in = nc.dram_tensor("bq_row", [1, 128], F32, kind="ExternalInput").ap()
    bkr_in = nc.dram_tensor("bk_row", [1, 128], F32, kind="ExternalInput").ap()
    bvr_in = nc.dram_tensor("bv_row", [1, 128], F32, kind="ExternalInput").ap()
    bor_in = nc.dram_tensor("bo_row", [1, D], F32, kind="ExternalInput").ap()
    b1a_in = nc.dram_tensor("b1a", [1, FFI], F32, kind="ExternalInput").ap()
    b1g_in = nc.dram_tensor("b1g", [32, 128, 1], F32, kind="ExternalInput").ap()
    b2r_in = nc.dram_tensor("b2_row", [1, D], F32, kind="ExternalInput").ap()

    q_out = nc.dram_tensor("q_out", [4, N, DH], F32, kind="ExternalOutput").ap()
    k_out = nc.dram_tensor("k_out", [4, N, DH], F32, kind="ExternalOutput").ap()
    v_out = nc.dram_tensor("v_out", [4, N, DH], F32, kind="ExternalOutput").ap()
    io_out = nc.dram_tensor("io_out", [4, N, DH], F32, kind="ExternalOutput").ap()
    xf_out = nc.dram_tensor("xf_out", [TOK, D], F32, kind="ExternalOutput").ap()
    x_out = nc.dram_tensor("x_out", [TPC, D], F32, kind="ExternalOutput").ap()

    with tile.TileContext(nc) as tc, \
         tc.tile_pool(name="const", bufs=1) as const, \
         tc.tile_pool(name="dram", bufs=1, space="DRAM") as dram:

        # ---- constants ----
        ident = const.tile([128, 128], F32R)
        ident_f = const.tile([128, 128], F32)
        make_identity(nc, ident_f[:])
        nc.scalar.copy(ident[:], ident_f[:])
        eps_t = const.tile([128, 1], F32)
        nc.vector.memset(eps_t[:], EPS)
        ones16 = const.tile([128, 16, 2], F32R)
        ones16_f = const.tile([128, 16, 2], F32)
        nc.vector.memset(ones16_f[:], 0.0)
        nc.vector.memset(ones16_f[:, :, 0:1], 1.0)
        nc.scalar.copy(ones16[:], ones16_f[:])
        onesrow = const.tile([1, 512], F32R)
        onesrow_f = const.tile([1, 512], F32)
        nc.vector.memset(onesrow_f[:], 1.0)
        nc.scalar.copy(onesrow[:], onesrow_f[:])

        # ---- resident weights ----
        brow = {}
        for name, src, has in (("q", bqr_in, has_bq), ("k", bkr_in, has_bk),
                               ("v", bvr_in, has_bv)):
            if has:
                bt = const.tile([1, 128], F32R, tag=f"brow_{name}")
                nc.gpsimd.dma_start(out=bt[:], in_=src)
                brow[name] = bt
        bo_t = b2_t = b1a_t = b1g_t = None
        if has_bo:
            bo_t = const.tile([1, D], F32R)
            nc.gpsimd.dma_start(out=bo_t[:], in_=bor_in)
        if has_b2:
            b2_t = const.tile([1, D], F32R)
            nc.gpsimd.dma_start(out=b2_t[:], in_=b2r_in)
        if has_b1a:
            b1a_t = const.tile([1, FFI], F32R)
            nc.gpsimd.dma_start(out=b1a_t[:], in_=b1a_in)
        if has_b1g:
            b1g_t = const.tile([128, 32], F32)
            nc.sync.dma_start(out=b1g_t[:],
                              in_=b1g_in.rearrange("m p one -> p (m one)"))

        # ---- collective buffers ----
        cc_in = dram.tile([TOK, 128], F32)
        cc_out = dram.tile([TOK, 128], F32)

        from contextlib import ExitStack
        qkv_es = ExitStack()
        with tc.tile_pool(name="attn_persist", bufs=1) as apers:
            QT = apers.tile([128, TOK], F32R)   # rows 0-63 head0, 64-127 head1
            KT = apers.tile([128, TOK], F32R)
            V_t = [apers.tile([128, 16, 66], F32R, tag=f"vtile{p}", name=f"vtile{p}")
                   for p in range(4)]
            qkvp = qkv_es.enter_context(tc.tile_pool(name="qkvw", bufs=1))
            VT = qkvp.tile([128, TOK], F32R)
            wq_t = qkvp.tile([128, KD, 128], F32R)
            wk_t = qkvp.tile([128, KD, 128], F32R)
            wv_t = qkvp.tile([128, KD, 128], F32R)
            nc.gpsimd.dma_start(out=wq_t[:], in_=wq_in)
            nc.gpsimd.dma_start(out=wk_t[:], in_=wk_in)
            nc.gpsimd.dma_start(out=wv_t[:], in_=wv_in)

            # ---------------- Phase 1: LN1 + transpose + QKV ----------------
            with tc.tile_pool(name="ph1", bufs=4) as p1, \
                 tc.tile_pool(name="ph1s", bufs=4) as p1s, \
                 tc.tile_pool(name="ph1xfT", bufs=2) as p1x, \
                 tc.tile_pool(name="ph1ps", bufs=3, space="PSUM") as p1ps, \
                 tc.tile_pool(name="ph1psq", bufs=3, space="PSUM") as p1psq:
                for ch in range(TOK // 512):
                    xn_tiles = []
                    for it in range(4):
                        row0 = 512 * ch + 128 * it
                        xt = p1.tile([128, D], F32, tag="xt")
                        nc.sync.dma_start(out=xt[:], in_=x_in[row0:row0 + 128, :])
                        rstd, nbias = _ln_stats(nc, p1s, xt[:], eps_t)
                        xn = p1.tile([128, D], F32R, tag="xn")
                        nc.scalar.activation(xn[:], xt[:], AF.Identity,
                                             bias=nbias[:], scale=rstd[:])
                        nc.sync.dma_start(out=xf_out[row0:row0 + 128, :],
                                          in_=xn[:].bitcast(F32))
                        xn_tiles.append(xn)
                    xfT = p1x.tile([128, KD, 512], F32R, tag="xfT")
                    for cb in range(KD):
                        tp = p1ps.tile([128, 4, 128], F32R, tag="tp")
                        for it in range(4):
                            nc.tensor.transpose(
                                tp[:, it, :],
                                xn_tiles[it][:, 128 * cb:128 * (cb + 1)],
                                ident[:])
                        nc.scalar.copy(xfT[:, cb, :],
                                       tp[:].rearrange("p a b -> p (a b)"))
                    for w_t, dst, bias_t in ((wq_t, QT, brow.get("q")),
                                             (wk_t, KT, brow.get("k")),
                                             (wv_t, VT, brow.get("v"))):
                        ps = p1psq.tile([128, 512], F32, tag="psqkv")
                        for k in range(KD):
                            nc.tensor.matmul(
                                ps[:], w_t[:, k, :], xfT[:, k, :],
                                start=(k == 0),
                                stop=(k == KD - 1 and bias_t is None))
                        if bias_t is not None:
                            nc.tensor.matmul(ps[:], bias_t[:], onesrow[:],
                                             start=False, stop=True)
                        nc.scalar.copy(dst[:, 512 * ch:512 * (ch + 1)], ps[:])

            # ---------------- Phase 2: q/k/v outputs + V tiles ----------------
            with tc.tile_pool(name="ph2ps", bufs=3, space="PSUM") as p2ps, \
                 tc.tile_pool(name="ph2sb", bufs=3) as p2sb:
                for pr, (b, hl) in enumerate(PAIRS):
                    bp, boff = 64 * hl, N * b
                    for src, kind in ((QT, "q"), (KT, "k"), (VT, "v")):
                        for g in range(4):
                            tq = p2ps.tile([128, 4, 64], F32R, tag="tq")
                            for tt in range(4):
                                col0 = boff + 128 * (4 * g + tt)
                                nc.tensor.transpose(
                                    tq[:, tt, :],
                                    src[bp:bp + 64, col0:col0 + 128],
                                    ident[bp:bp + 64, bp:bp + 64])
                            if kind == "v":
                                nc.vector.tensor_copy(
                                    V_t[pr][:, 4 * g:4 * g + 4, 0:64], tq[:])
                            else:
                                stage = p2sb.tile([128, 4, 64], F32, tag="stage")
                                nc.vector.tensor_copy(stage[:], tq[:])
                                dst = q_out if kind == "q" else k_out
                                nc.sync.dma_start(
                                    out=dst[pr, 512 * g:512 * (g + 1), :]
                                    .rearrange("(t p) d -> p t d", p=128),
                                    in_=stage[:])
                    nc.scalar.copy(V_t[pr][:, :, 64:66], ones16[:])
                    nc.sync.dma_start(
                        out=v_out[pr].rearrange("(j p) d -> p j d", p=128),
                        in_=V_t[pr][:, :, 0:64].bitcast(F32))

            qkv_es.close()

            # ---------------- Phase 3: attention ----------------
            with tc.tile_pool(name="ph3et", bufs=2) as p3et, \
                 tc.tile_pool(name="ph3ps", bufs=3, space="PSUM") as p3ps, \
                 tc.tile_pool(name="ph3av", bufs=2, space="PSUM") as p3av, \
                 tc.tile_pool(name="ph3tr", bufs=2, space="PSUM") as p3tr, \
                 tc.tile_pool(name="ph3sb", bufs=2) as p3sb:
                for pr, (b, hl) in enumerate(PAIRS):
                    bp, boff = 64 * hl, N * b
                    for ic in range(4):
                        i0 = boff + 512 * ic
                        et = p3et.tile([128, 16, 512], F32R, tag="et")
                        ps_av = p3av.tile([66, 512], F32, tag="psav")
                        for j in range(16):
                            ps_st = p3ps.tile([128, 512], F32, tag="psst")
                            nc.tensor.matmul(
                                ps_st[:],
                                KT[bp:bp + 64,
                                   boff + 128 * j:boff + 128 * (j + 1)],
                                QT[bp:bp + 64, i0:i0 + 512],
                                start=True, stop=True)
                            nc.scalar.activation(et[:, j, :], ps_st[:], AF.Exp,
                                                 scale=float(DH) ** -0.5)
                            nc.tensor.matmul(ps_av[:], V_t[pr][:, j, :],
                                             et[:, j, :],
                                             start=(j == 0), stop=(j == 15))
                        ot_sb = p3sb.tile([66, 512], F32R, tag="otsb")
                        nc.scalar.copy(ot_sb[:], ps_av[:])
                        o_sb = p3sb.tile([128, 4, 64], F32, tag="osb")
                        for tt in range(4):
                            tr = p3tr.tile([128, 66], F32R, tag="tr3")
                            nc.tensor.transpose(
                                tr[:, 0:66],
                                ot_sb[0:66, 128 * tt:128 * (tt + 1)],
                                ident[0:66, 0:66])
                            rcp = p3sb.tile([128, 1], F32, tag="rcp")
                            nc.vector.reciprocal(rcp[:], tr[:, 64:65])
                            nc.vector.tensor_scalar_mul(o_sb[:, tt, :],
                                                        tr[:, 0:64], rcp[:])
                        nc.sync.dma_start(
                            out=io_out[pr, 512 * ic:512 * (ic + 1), :]
                            .rearrange("(t p) d -> p t d", p=128),
                            in_=o_sb[:])
                        nc.sync.dma_start(
                            out=cc_in[boff + 512 * ic:boff + 512 * (ic + 1),
                                      64 * hl:64 * (hl + 1)]
                            .rearrange("(t p) d -> p t d", p=128),
                            in_=o_sb[:])

        # ---------------- Phase 4: AllToAll exchange ----------------
        nc.gpsimd.collective_compute(
            "AllToAll", ALU.bypass,
            replica_groups=[list(range(N_CORES))],
            ins=[cc_in[:].opt()], outs=[cc_out[:].opt()],
        )

        # -------- Phase 5: Wo + residual + LN3 + GEGLU FF --------
        with tc.tile_pool(name="ph5y", bufs=1) as p5y, \
             tc.tile_pool(name="ph5s", bufs=4) as p5s, \
             tc.tile_pool(name="ph5big", bufs=1) as p5big:
            y_sb = p5y.tile([128, 4, D], F32)
            h3n_tiles = [p5s.tile([128, D], F32R, tag=f"h3n{tt}")
                         for tt in range(4)]
            with tc.tile_pool(name="ph5osl", bufs=1) as p5o, \
                 tc.tile_pool(name="ph5ps", bufs=2, space="PSUM") as p5ps:
                wo_t = p5o.tile([128, KD, D], F32R)
                nc.gpsimd.dma_start(out=wo_t[:], in_=wo_in)
                osl = [p5o.tile([128, 8, 128], F32R, tag=f"osl{tt}", name=f"osl{tt}")
                       for tt in range(4)]
                for j in range(8):
                    for tt in range(4):
                        nc.gpsimd.dma_start(
                            out=osl[tt][:, j, :],
                            in_=cc_out[512 * j + 128 * tt:
                                       512 * j + 128 * (tt + 1), :])
                OTt = p5o.tile([128, KD, 512], F32R)
                for j in range(8):
                    tro = p5ps.tile([128, 4, 128], F32R, tag="tro")
                    for tt in range(4):
                        nc.tensor.transpose(tro[:, tt, :], osl[tt][:, j, :],
                                            ident[:])
                    nc.scalar.copy(OTt[:, j, :],
                                   tro[:].rearrange("p a b -> p (a b)"))

                xres_t = p5y.tile([128, 4, D], F32, tag="xres")
                nc.sync.dma_start(
                    out=xres_t[:],
                    in_=xres_in.rearrange("(t p) d -> p t d", p=128))
                for tt in range(4):
                    ps_y = p5ps.tile([128, D], F32, tag="psy")
                    for nch in range(2):
                        for k in range(KD):
                            nc.tensor.matmul(
                                ps_y[:, 512 * nch:512 * (nch + 1)],
                                OTt[:, k, 128 * tt:128 * (tt + 1)],
                                wo_t[:, k, 512 * nch:512 * (nch + 1)],
                                start=(k == 0),
                                stop=(k == KD - 1 and not has_bo))
                        if has_bo:
                            nc.tensor.matmul(
                                ps_y[:, 512 * nch:512 * (nch + 1)],
                                onesrow[:, 0:128],
                                bo_t[:, 512 * nch:512 * (nch + 1)],
                                start=False, stop=True)
                    nc.vector.tensor_tensor(y_sb[:, tt, :], ps_y[:],
                                            xres_t[:, tt, :], op=ALU.add)
                    rstd3, nbias3 = _ln_stats(nc, p5s, y_sb[:, tt, :], eps_t)
                    nc.scalar.activation(h3n_tiles[tt][:], y_sb[:, tt, :],
                                         AF.Identity,
                                         bias=nbias3[:], scale=rstd3[:])

            # osl/OTt/wo freed; build h3T, then FF
            big_es = ExitStack()
            p5big = big_es.enter_context(tc.tile_pool(name="ph5big", bufs=1))
            h3T = p5big.tile([128, KD, 512], F32R)
            zT = p5big.tile([128, 32, 512], F32R)
            with tc.tile_pool(name="ph5pst", bufs=2, space="PSUM") as p5pst:
                for cb in range(KD):
                    tr5 = p5pst.tile([128, 4, 128], F32R, tag="tr5")
                    for tt in range(4):
                        nc.tensor.transpose(
                            tr5[:, tt, :],
                            h3n_tiles[tt][:, 128 * cb:128 * (cb + 1)],
                            ident[:])
                    nc.scalar.copy(h3T[:, cb, :],
                                   tr5[:].rearrange("p a b -> p (a b)"))

            with tc.tile_pool(name="ph5w1", bufs=3) as p5w1, \
                 tc.tile_pool(name="ph5ff1", bufs=3, space="PSUM") as p5f1, \
                 tc.tile_pool(name="ph5gel", bufs=3) as p5g:
                for m in range(32):
                    w1a = p5w1.tile([128, KD, 128], F32R, tag="w1a")
                    nc.gpsimd.dma_start(out=w1a[:], in_=w1_in[m])
                    w1g = p5w1.tile([128, KD, 128], F32R, tag="w1g")
                    nc.gpsimd.dma_start(out=w1g[:], in_=w1_in[m + 32])
                    psA = p5f1.tile([128, 512], F32, tag="psA")
                    psG = p5f1.tile([128, 512], F32, tag="psG")
                    for k in range(KD):
                        nc.tensor.matmul(psA[:], w1a[:, k, :], h3T[:, k, :],
                                         start=(k == 0),
                                         stop=(k == KD - 1 and not has_b1a))
                        nc.tensor.matmul(psG[:], w1g[:, k, :], h3T[:, k, :],
                                         start=(k == 0), stop=(k == KD - 1))
                    if has_b1a:
                        nc.tensor.matmul(psA[:],
                                         b1a_t[:, 128 * m:128 * (m + 1)],
                                         onesrow[:], start=False, stop=True)
                    gel = p5g.tile([128, 512], F32, tag="gel")
                    if has_b1g:
                        nc.scalar.activation(gel[:], psG[:], AF.Gelu,
                                             bias=b1g_t[:, m:m + 1])
                    else:
                        nc.scalar.activation(gel[:], psG[:], AF.Gelu)
                    nc.vector.tensor_mul(zT[:, m, :], psA[:], gel[:])

            with tc.tile_pool(name="ph5w2", bufs=6) as p5w2, \
                 tc.tile_pool(name="ph5ff2", bufs=1, space="PSUM") as p5f2:
                ps2 = [p5f2.tile([128, 512], F32, tag=f"ps2_{i}", name=f"ps2_{i}")
                       for i in range(8)]
                for kt in range(32):
                    w2 = p5w2.tile([128, D], F32R, tag="w2")
                    nc.gpsimd.dma_start(out=w2[:], in_=w2_in[kt])
                    for tt in range(4):
                        for nch in range(2):
                            nc.tensor.matmul(
                                ps2[tt * 2 + nch][:],
                                zT[:, kt, 128 * tt:128 * (tt + 1)],
                                w2[:, 512 * nch:512 * (nch + 1)],
                                start=(kt == 0),
                                stop=(kt == 31 and not has_b2))
                if has_b2:
                    for tt in range(4):
                        for nch in range(2):
                            nc.tensor.matmul(ps2[tt * 2 + nch][:],
                                             onesrow[:, 0:128],
                                             b2_t[:, 512 * nch:512 * (nch + 1)],
                                             start=False, stop=True)
                xo = p5y.tile([128, 4, D], F32, tag="xres")  # reuse xres slot
                for tt in range(4):
                    for nch in range(2):
                        nc.vector.tensor_tensor(
                            xo[:, tt, 512 * nch:512 * (nch + 1)],
                            ps2[tt * 2 + nch][:],
                            y_sb[:, tt, 512 * nch:512 * (nch + 1)],
                            op=ALU.add)
                nc.sync.dma_start(
                    out=x_out.rearrange("(t p) d -> p t d", p=128),
                    in_=xo[:])
            big_es.close()

    nc.compile()
    return nc


_CACHE = {}


def kernel(**inputs):
    x = np.asarray(inputs["x"], np.float32).reshape(TOK, D)
    Wq, Wk, Wv = (np.asarray(inputs[n], np.float32) for n in ("Wq", "Wk", "Wv"))
    Wo = np.asarray(inputs["Wo"], np.float32)
    bo = np.asarray(inputs["bo"], np.float32)
    g1, b1 = (np.asarray(inputs["ln1_g"], np.float32),
              np.asarray(inputs["ln1_b"], np.float32))
    g3, b3 = (np.asarray(inputs["ln3_g"], np.float32),
              np.asarray(inputs["ln3_b"], np.float32))
    Wff1 = np.asarray(inputs["Wff1"], np.float32)
    bff1 = np.asarray(inputs["bff1"], np.float32)
    Wff2 = np.asarray(inputs["Wff2"], np.float32)
    bff2 = np.asarray(inputs["bff2"], np.float32)

    Wq_e, Wk_e, Wv_e = g1[:, None] * Wq, g1[:, None] * Wk, g1[:, None] * Wv
    bq_full, bk_full, bv_full = b1 @ Wq, b1 @ Wk, b1 @ Wv
    W1_e = g3[:, None] * Wff1
    b1_full = b3 @ Wff1 + bff1
    b1a_full, b1g_full = b1_full[:FFI], b1_full[FFI:]
    key = (bool(np.any(bq_full)), bool(np.any(bk_full)), bool(np.any(bv_full)),
           bool(np.any(bo)), bool(np.any(b1a_full)), bool(np.any(b1g_full)),
           bool(np.any(bff2)))
    if key not in _CACHE:
        _CACHE[key] = build_program(*key)
    nc = _CACHE[key]

    wo_h = Wo.reshape(KD, 128, D).transpose(1, 0, 2).copy()
    w1_h = W1_e.reshape(KD, 128, 64, 128).transpose(2, 1, 0, 3).copy()
    w2_h = Wff2.reshape(32, 128, D).copy()

    in_maps = []
    for c in range(N_CORES):
        cs = slice(128 * c, 128 * (c + 1))

        def tile_w(W):
            return W[:, cs].reshape(KD, 128, 128).transpose(1, 0, 2).copy()

        in_maps.append({
            "x": x,
            "x_res": x[TPC * c:TPC * (c + 1)].copy(),
            "wq": tile_w(Wq_e), "wk": tile_w(Wk_e), "wv": tile_w(Wv_e),
            "wo": wo_h, "wff1": w1_h, "wff2": w2_h,
            "bq_row": bq_full[None, cs].copy(),
            "bk_row": bk_full[None, cs].copy(),
            "bv_row": bv_full[None, cs].copy(),
            "bo_row": bo[None, :].copy(),
            "b1a": b1a_full[None, :].copy(),
            "b1g": b1g_full.reshape(32, 128, 1).copy(),
            "b2_row": bff2[None, :].copy(),
        })

    res = bass_utils.run_bass_kernel_spmd(
        nc, in_maps, core_ids=list(range(N_CORES)),
        trace=bool(int(os.environ.get("KBENCH_TRACE", "0"))))
    kernel.last_results = res

    xq = np.empty((B * H, N, DH), np.float32)
    xk = np.empty((B * H, N, DH), np.float32)
    xv = np.empty((B * H, N, DH), np.float32)
    io = np.empty((B, H, N, DH), np.float32)
    xfull = np.empty((TOK, D), np.float32)
    for c in range(N_CORES):
        r = res.results[c]
        for pr, (b, hl) in enumerate(PAIRS):
            h = 2 * c + hl
            xq[b * H + h] = r["q_out"][pr]
            xk[b * H + h] = r["k_out"][pr]
            xv[b * H + h] = r["v_out"][pr]
            io[b, h] = r["io_out"][pr]
        xfull[TPC * c:TPC * (c + 1)] = r["x_out"]
    xf = res.results[0]["xf_out"]
    if np.any(g1 != 1.0) or np.any(b1 != 0.0):
        xf = xf * g1 + b1
    return (xfull.reshape(B, N, D), xq, xk, xv, io,
            xf.reshape(B, N, D).astype(np.float32))


# revision 9
# speedup vs baseline: 1.0919x; 1.0919x over previous
"""Bass/Tile TRN2 kernel for nn_BasicTransformerBlock (B=2,N=2048,D=1024,H=16,DH=64,GEGLU).

Sharding: 8 cores, 2 heads/core for attention (head-parallel), token-sharded
Wo+FF (512 tokens/core) after an AllToAll exchange of the attention output.
All matmuls in float32r (full PE rate, ~1e-4 rounding).
"""
import os
import numpy as np
import concourse.bass as bass
import concourse.tile as tile
from concourse import bacc, mybir
from concourse import bass_utils
from concourse.masks import make_identity

F32 = mybir.dt.float32
F32R = mybir.dt.float32r
AF = mybir.ActivationFunctionType
ALU = mybir.AluOpType

N_CORES = 8
B, N, D, H, DH = 2, 2048, 1024, 16, 64
TOK = B * N                    # 4096
FFI = 4 * D                    # 4096
EPS = 1e-5
TPC = TOK // N_CORES           # 512 tokens per core (consumer side)
KD = D // 128                  # 8 k-tiles over D
PAIRS = [(b, hl) for b in range(B) for hl in range(2)]   # 4 (batch, local-head) pairs


def _ln_stats(nc, pool, src_ap, eps_t):
    """Row LayerNorm stats of src_ap [128, 1024] -> (rstd [128,1], nbias [128,1])."""
    st = pool.tile([128, 2, 6], F32, tag="lnst")
    nc.vector.bn_stats(out=st[:, 0, :], in_=src_ap[:, 0:512])
    nc.vector.bn_stats(out=st[:, 1, :], in_=src_ap[:, 512:1024])
    mv = pool.tile([128, 2], F32, tag="lnmv")
    nc.vector.bn_aggr(out=mv[:], in_=st[:])
    lnv = pool.tile([128, 1], F32, tag="lnv")
    nc.vector.tensor_scalar_add(lnv[:], mv[:, 1:2], EPS)
    rcpv = pool.tile([128, 1], F32, tag="lnrcp")
    nc.vector.reciprocal(rcpv[:], lnv[:])
    rstd = pool.tile([128, 1], F32, tag="lnrstd")
    nc.scalar.activation(rstd[:], rcpv[:], AF.Sqrt)
    nbias = pool.tile([128, 1], F32, tag="lnnb")
    nc.vector.tensor_scalar(nbias[:], mv[:, 0:1], rstd[:], -1.0,
                            op0=ALU.mult, op1=ALU.mult)
    return rstd, nbias


def build_program(has_bq, has_bk, has_bv, has_bo, has_b1a, has_b1g, has_b2):
    nc = bacc.Bacc("TRN2", target_bir_lowering=False, debug=False,
                   num_devices=N_CORES)

    x_in = nc.dram_tensor("x", [TOK, D], F32, kind="ExternalInput").ap()
    xres_in = nc.dram_tensor("x_res", [TPC, D], F32, kind="ExternalInput").ap()
    wq_in = nc.dram_tensor("wq", [128, KD, 128], F32, kind="ExternalInput").ap()
    wk_in = nc.dram_tensor("wk", [128, KD, 128], F32, kind="ExternalInput").ap()
    wv_in = nc.dram_tensor("wv", [128, KD, 128], F32, kind="ExternalInput").ap()
    wo_in = nc.dram_tensor("wo", [128, KD, D], F32, kind="ExternalInput").ap()
    w1_in = nc.dram_tensor("wff1", [64, 128, KD, 128], F32, kind="ExternalInput").ap()
    w2_in = nc.dram_tensor("wff2", [32, 128, D], F32, kind="ExternalInput").ap()
    bqr_in = nc.dram_tensor("bq_row", [1, 128], F32, kind="ExternalInput").ap()
    bkr_in = nc.dram_tensor("bk_row", [1, 128], F32, kind="ExternalInput").ap()
    bvr_in = nc.dram_tensor("bv_row", [1, 128], F32, kind="ExternalInput").ap()
    bor_in = nc.dram_tensor("bo_row", [1, D], F32, kind="ExternalInput").ap()
    b1a_in = nc.dram_tensor("b1a", [1, FFI], F32, kind="ExternalInput").ap()
    b1g_in = nc.dram_tensor("b1g", [32, 128, 1], F32, kind="ExternalInput").ap()
    b2r_in = nc.dram_tensor("b2_row", [1, D], F32, kind="ExternalInput").ap()

    q_out = nc.dram_tensor("q_out", [4, N, DH], F32, kind="ExternalOutput").ap()
    k_out = nc.dram_tensor("k_out", [4, N, DH], F32, kind="ExternalOutput").ap()
    v_out = nc.dram_tensor("v_out", [4, N, DH], F32, kind="ExternalOutput").ap()
    io_out = nc.dram_tensor("io_out", [4, N, DH], F32, kind="ExternalOutput").ap()
    xf_out = nc.dram_tensor("xf_out", [TOK, D], F32, kind="ExternalOutput").ap()
    x_out = nc.dram_tensor("x_out", [TPC, D], F32, kind="ExternalOutput").ap()

    with tile.TileContext(nc) as tc, \
         tc.tile_pool(name="const", bufs=1) as const, \
         tc.tile_pool(name="dram", bufs=1, space="DRAM") as dram:

        # ---- constants ----
        ident = const.tile([128, 128], F32R)
        ident_f = const.tile([128, 128], F32)
        make_identity(nc, ident_f[:])
        nc.scalar.copy(ident[:], ident_f[:])
        eps_t = const.tile([128, 1], F32)
        nc.vector.memset(eps_t[:], EPS)
        ones16 = const.tile([128, 16, 2], F32R)
        ones16_f = const.tile([128, 16, 2], F32)
        nc.vector.memset(ones16_f[:], 0.0)
        nc.vector.memset(ones16_f[:, :, 0:1], 1.0)
        nc.scalar.copy(ones16[:], ones16_f[:])
        onesrow = const.tile([1, 512], F32R)
        onesrow_f = const.tile([1, 512], F32)
        nc.vector.memset(onesrow_f[:], 1.0)
        nc.scalar.copy(onesrow[:], onesrow_f[:])

        # ---- resident weights ----
        brow = {}
        for name, src, has in (("q", bqr_in, has_bq), ("k", bkr_in, has_bk),
                               ("v", bvr_in, has_bv)):
            if has:
                bt = const.tile([1, 128], F32R, tag=f"brow_{name}")
                nc.gpsimd.dma_start(out=bt[:], in_=src)
                brow[name] = bt
        bo_t = b2_t = b1a_t = b1g_t = None
        if has_bo:
            bo_t = const.tile([1, D], F32R)
            nc.gpsimd.dma_start(out=bo_t[:], in_=bor_in)
        if has_b2:
            b2_t = const.tile([1, D], F32R)
            nc.gpsimd.dma_start(out=b2_t[:], in_=b2r_in)
        if has_b1a:
            b1a_t = const.tile([1, FFI], F32R)
            nc.gpsimd.dma_start(out=b1a_t[:], in_=b1a_in)
        if has_b1g:
            b1g_t = const.tile([128, 32], F32)
            nc.sync.dma_start(out=b1g_t[:],
                              in_=b1g_in.rearrange("m p one -> p (m one)"))

        # ---- collective buffers ----
        cc_in = dram.tile([TOK, 128], F32)
        cc_out = dram.tile([TOK, 128], F32)

        from contextlib import ExitStack
        qkv_es = ExitStack()
        with tc.tile_pool(name="attn_persist", bufs=1) as apers:
            QT = apers.tile([128, TOK], F32R)   # rows 0-63 head0, 64-127 head1
            KT = apers.tile([128, TOK], F32R)
            V_t = [apers.tile([128, 16, 66], F32R, tag=f"vtile{p}", name=f"vtile{p}")
                   for p in range(4)]
            qkvp = qkv_es.enter_context(tc.tile_pool(name="qkvw", bufs=1))
            VT = qkvp.tile([128, TOK], F32R)
            wq_t = qkvp.tile([128, KD, 128], F32R)
            wk_t = qkvp.tile([128, KD, 128], F32R)
            wv_t = qkvp.tile([128, KD, 128], F32R)
            nc.gpsimd.dma_start(out=wq_t[:], in_=wq_in)
            nc.gpsimd.dma_start(out=wk_t[:], in_=wk_in)
            nc.gpsimd.dma_start(out=wv_t[:], in_=wv_in)

            # ---------------- Phase 1: LN1 + transpose + QKV ----------------
            with tc.tile_pool(name="ph1", bufs=4) as p1, \
                 tc.tile_pool(name="ph1s", bufs=4) as p1s, \
                 tc.tile_pool(name="ph1xfT", bufs=2) as p1x, \
                 tc.tile_pool(name="ph1ps", bufs=3, space="PSUM") as p1ps, \
                 tc.tile_pool(name="ph1psq", bufs=3, space="PSUM") as p1psq:
                for ch in range(TOK // 512):
                    xn_tiles = []
                    for it in range(4):
                        row0 = 512 * ch + 128 * it
                        xt = p1.tile([128, D], F32, tag="xt")
                        nc.sync.dma_start(out=xt[:], in_=x_in[row0:row0 + 128, :])
                        rstd, nbias = _ln_stats(nc, p1s, xt[:], eps_t)
                        xn = p1.tile([128, D], F32R, tag="xn")
                        nc.scalar.activation(xn[:], xt[:], AF.Identity,
                                             bias=nbias[:], scale=rstd[:])
                        nc.sync.dma_start(out=xf_out[row0:row0 + 128, :],
                                          in_=xn[:].bitcast(F32))
                        xn_tiles.append(xn)
                    xfT = p1x.tile([128, KD, 512], F32R, tag="xfT")
                    for cb in range(KD):
                        tp = p1ps.tile([128, 4, 128], F32R, tag="tp")
                        for it in range(4):
                            nc.tensor.transpose(
                                tp[:, it, :],
                                xn_tiles[it][:, 128 * cb:128 * (cb + 1)],
                                ident[:])
                        nc.vector.tensor_copy(xfT[:, cb, :],
                                              tp[:].rearrange("p a b -> p (a b)"))
                    for w_t, dst, bias_t in ((wq_t, QT, brow.get("q")),
                                             (wk_t, KT, brow.get("k")),
                                             (wv_t, VT, brow.get("v"))):
                        ps = p1psq.tile([128, 512], F32, tag="psqkv")
                        for k in range(KD):
                            nc.tensor.matmul(
                                ps[:], w_t[:, k, :], xfT[:, k, :],
                                start=(k == 0),
                                stop=(k == KD - 1 and bias_t is None))
                        if bias_t is not None:
                            nc.tensor.matmul(ps[:], bias_t[:], onesrow[:],
                                             start=False, stop=True)
                        nc.vector.tensor_copy(dst[:, 512 * ch:512 * (ch + 1)], ps[:])

            # ---------------- Phase 2: q/k/v outputs + V tiles ----------------
            with tc.tile_pool(name="ph2ps", bufs=3, space="PSUM") as p2ps, \
                 tc.tile_pool(name="ph2sb", bufs=3) as p2sb:
                for pr, (b, hl) in enumerate(PAIRS):
                    bp, boff = 64 * hl, N * b
                    for src, kind in ((QT, "q"), (KT, "k"), (VT, "v")):
                        for g in range(4):
                            tq = p2ps.tile([128, 4, 64], F32R, tag="tq")
                            for tt in range(4):
                                col0 = boff + 128 * (4 * g + tt)
                                nc.tensor.transpose(
                                    tq[:, tt, :],
                                    src[bp:bp + 64, col0:col0 + 128],
                                    ident[bp:bp + 64, bp:bp + 64])
                            if kind == "v":
                                nc.vector.tensor_copy(
                                    V_t[pr][:, 4 * g:4 * g + 4, 0:64], tq[:])
                            else:
                                stage = p2sb.tile([128, 4, 64], F32, tag="stage")
                                nc.vector.tensor_copy(stage[:], tq[:])
                                dst = q_out if kind == "q" else k_out
                                nc.sync.dma_start(
                                    out=dst[pr, 512 * g:512 * (g + 1), :]
                                    .rearrange("(t p) d -> p t d", p=128),
                                    in_=stage[:])
                    nc.scalar.copy(V_t[pr][:, :, 64:66], ones16[:])
                    nc.sync.dma_start(
                        out=v_out[pr].rearrange("(j p) d -> p j d", p=128),
                        in_=V_t[pr][:, :, 0:64].bitcast(F32))

            qkv_es.close()

            # ---------------- Phase 3: attention ----------------
            with tc.tile_pool(name="ph3et", bufs=2) as p3et, \
                 tc.tile_pool(name="ph3ps", bufs=3, space="PSUM") as p3ps, \
                 tc.tile_pool(name="ph3av", bufs=2, space="PSUM") as p3av, \
                 tc.tile_pool(name="ph3tr", bufs=2, space="PSUM") as p3tr, \
                 tc.tile_pool(name="ph3sb", bufs=2) as p3sb:
                for pr, (b, hl) in enumerate(PAIRS):
                    bp, boff = 64 * hl, N * b
                    for ic in range(4):
                        i0 = boff + 512 * ic
                        et = p3et.tile([128, 16, 512], F32R, tag="et")
                        ps_av = p3av.tile([66, 512], F32, tag="psav")
                        for j in range(16):
                            ps_st = p3ps.tile([128, 512], F32, tag="psst")
                            nc.tensor.matmul(
                                ps_st[:],
                                KT[bp:bp + 64,
                                   boff + 128 * j:boff + 128 * (j + 1)],
                                QT[bp:bp + 64, i0:i0 + 512],
                                start=True, stop=True)
                            nc.scalar.activation(et[:, j, :], ps_st[:], AF.Exp,
                                                 scale=float(DH) ** -0.5)
                            nc.tensor.matmul(ps_av[:], V_t[pr][:, j, :],
                                             et[:, j, :],
                                             start=(j == 0), stop=(j == 15))
                        ot_sb = p3sb.tile([66, 512], F32R, tag="otsb")
                        nc.scalar.copy(ot_sb[:], ps_av[:])
                        o_sb = p3sb.tile([128, 4, 64], F32, tag="osb")
                        for tt in range(4):
                            tr = p3tr.tile([128, 66], F32R, tag="tr3")
                            nc.tensor.transpose(
                                tr[:, 0:66],
                                ot_sb[0:66, 128 * tt:128 * (tt + 1)],
                                ident[0:66, 0:66])
                            rcp = p3sb.tile([128, 1], F32, tag="rcp")
                            nc.vector.reciprocal(rcp[:], tr[:, 64:65])
                            nc.vector.tensor_scalar_mul(o_sb[:, tt, :],
                                                        tr[:, 0:64], rcp[:])
                        nc.sync.dma_start(
                            out=io_out[pr, 512 * ic:512 * (ic + 1), :]
                            .rearrange("(t p) d -> p t d", p=128),
                            in_=o_sb[:])
                        nc.sync.dma_start(
                            out=cc_in[boff + 512 * ic:boff + 512 * (ic + 1),
                                      64 * hl:64 * (hl + 1)]
                            .rearrange("(t p) d -> p t d", p=128),
                            in_=o_sb[:])

        # ---------------- Phase 4: AllToAll exchange ----------------
        nc.gpsimd.collective_compute(
            "AllToAll", ALU.bypass,
            replica_groups=[list(range(N_CORES))],
            ins=[cc_in[:].opt()], outs=[cc_out[:].opt()],
        )

        # -------- Phase 5: Wo + residual + LN3 + GEGLU FF --------
        with tc.tile_pool(name="ph5y", bufs=1) as p5y, \
             tc.tile_pool(name="ph5s", bufs=4) as p5s, \
             tc.tile_pool(name="ph5big", bufs=1) as p5big:
            y_sb = p5y.tile([128, 4, D], F32)
            h3n_tiles = [p5s.tile([128, D], F32R, tag=f"h3n{tt}")
                         for tt in range(4)]
            with tc.tile_pool(name="ph5osl", bufs=1) as p5o, \
                 tc.tile_pool(name="ph5ps", bufs=2, space="PSUM") as p5ps:
                wo_t = p5o.tile([128, KD, D], F32R)
                nc.gpsimd.dma_start(out=wo_t[:], in_=wo_in)
                osl = [p5o.tile([128, 8, 128], F32R, tag=f"osl{tt}", name=f"osl{tt}")
                       for tt in range(4)]
                for j in range(8):
                    for tt in range(4):
                        nc.gpsimd.dma_start(
                            out=osl[tt][:, j, :],
                            in_=cc_out[512 * j + 128 * tt:
                                       512 * j + 128 * (tt + 1), :])
                OTt = p5o.tile([128, KD, 512], F32R)
                for j in range(8):
                    tro = p5ps.tile([128, 4, 128], F32R, tag="tro")
                    for tt in range(4):
                        nc.tensor.transpose(tro[:, tt, :], osl[tt][:, j, :],
                                            ident[:])
                    nc.scalar.copy(OTt[:, j, :],
                                   tro[:].rearrange("p a b -> p (a b)"))

                xres_t = p5y.tile([128, 4, D], F32, tag="xres")
                nc.sync.dma_start(
                    out=xres_t[:],
                    in_=xres_in.rearrange("(t p) d -> p t d", p=128))
                for tt in range(4):
                    ps_y = p5ps.tile([128, D], F32, tag="psy")
                    for nch in range(2):
                        for k in range(KD):
                            nc.tensor.matmul(
                                ps_y[:, 512 * nch:512 * (nch + 1)],
                                OTt[:, k, 128 * tt:128 * (tt + 1)],
                                wo_t[:, k, 512 * nch:512 * (nch + 1)],
                                start=(k == 0),
                                stop=(k == KD - 1 and not has_bo))
                        if has_bo:
                            nc.tensor.matmul(
                                ps_y[:, 512 * nch:512 * (nch + 1)],
                                onesrow[:, 0:128],
                                bo_t[:, 512 * nch:512 * (nch + 1)],
                                start=False, stop=True)
                    nc.vector.tensor_tensor(y_sb[:, tt, :], ps_y[:],
                                            xres_t[:, tt, :], op=ALU.add)
                    rstd3, nbias3 = _ln_stats(nc, p5s, y_sb[:, tt, :], eps_t)
                    nc.scalar.activation(h3n_tiles[tt][:], y_sb[:, tt, :],
                                         AF.Identity,
                                         bias=nbias3[:], scale=rstd3[:])

            # osl/OTt/wo freed; build h3T, then FF
            big_es = ExitStack()
            p5big = big_es.enter_context(tc.tile_pool(name="ph5big", bufs=1))
            h3T = p5big.tile([128, KD, 512], F32R)
            zT = p5big.tile([128, 32, 512], F32R)
            with tc.tile_pool(name="ph5pst", bufs=2, space="PSUM") as p5pst:
                for cb in range(KD):
                    tr5 = p5pst.tile([128, 4, 128], F32R, tag="tr5")
                    for tt in range(4):
                        nc.tensor.transpose(
                            tr5[:, tt, :],
                            h3n_tiles[tt][:, 128 * cb:128 * (cb + 1)],
                            ident[:])
                    nc.scalar.copy(h3T[:, cb, :],
                                   tr5[:].rearrange("p a b -> p (a b)"))

            with tc.tile_pool(name="ph5w1", bufs=3) as p5w1, \
                 tc.tile_pool(name="ph5ff1", bufs=3, space="PSUM") as p5f1, \
                 tc.tile_pool(name="ph5gel", bufs=3) as p5g:
                for m in range(32):
                    w1a = p5w1.tile([128, KD, 128], F32R, tag="w1a")
                    nc.gpsimd.dma_start(out=w1a[:], in_=w1_in[m])
                    w1g = p5w1.tile([128, KD, 128], F32R, tag="w1g")
                    nc.gpsimd.dma_start(out=w1g[:], in_=w1_in[m + 32])
                    psA = p5f1.tile([128, 512], F32, tag="psA")
                    psG = p5f1.tile([128, 512], F32, tag="psG")
                    for k in range(KD):
                        nc.tensor.matmul(psA[:], w1a[:, k, :], h3T[:, k, :],
                                         start=(k == 0),
                                         stop=(k == KD - 1 and not has_b1a))
                        nc.tensor.matmul(psG[:], w1g[:, k, :], h3T[:, k, :],
                                         start=(k == 0), stop=(k == KD - 1))
                    if has_b1a:
                        nc.tensor.matmul(psA[:],
                                         b1a_t[:, 128 * m:128 * (m + 1)],
                                         onesrow[:], start=False, stop=True)
                    gel = p5g.tile([128, 512], F32, tag="gel")
                    if has_b1g:
                        nc.scalar.activation(gel[:], psG[:], AF.Gelu,
                                             bias=b1g_t[:, m:m + 1])
                    else:
                        nc.scalar.activation(gel[:], psG[:], AF.Gelu)
                    nc.vector.tensor_mul(zT[:, m, :], psA[:], gel[:])

            with tc.tile_pool(name="ph5w2", bufs=6) as p5w2, \
                 tc.tile_pool(name="ph5ff2", bufs=1, space="PSUM") as p5f2:
                ps2 = [p5f2.tile([128, 512], F32, tag=f"ps2_{i}", name=f"ps2_{i}")
                       for i in range(8)]
                for kt in range(32):
                    w2 = p5w2.tile([128, D], F32R, tag="w2")
                    nc.gpsimd.dma_start(out=w2[:], in_=w2_in[kt])
                    for tt in range(4):
                        for nch in range(2):
                            nc.tensor.matmul(
                                ps2[tt * 2 + nch][:],
                                zT[:, kt, 128 * tt:128 * (tt + 1)],
                                w2[:, 512 * nch:512 * (nch + 1)],
                                start=(kt == 0),
                                stop=(kt == 31 and not has_b2))
                if has_b2:
                    for tt in range(4):
                        for nch in range(2):
                            nc.tensor.matmul(ps2[tt * 2 + nch][:],
                                             onesrow[:, 0:128],
                                             b2_t[:, 512 * nch:512 * (nch + 1)],
                                             start=False, stop=True)
                xo = p5y.tile([128, 4, D], F32, tag="xres")  # reuse xres slot
                for tt in range(4):
                    for nch in range(2):
                        nc.vector.tensor_tensor(
                            xo[:, tt, 512 * nch:512 * (nch + 1)],
                            ps2[tt * 2 + nch][:],
                            y_sb[:, tt, 512 * nch:512 * (nch + 1)],
                            op=ALU.add)
                nc.sync.dma_start(
                    out=x_out.rearrange("(t p) d -> p t d", p=128),
                    in_=xo[:])
            big_es.close()

    nc.compile()
    return nc


_CACHE = {}


def kernel(**inputs):
    x = np.asarray(inputs["x"], np.float32).reshape(TOK, D)
    Wq, Wk, Wv = (np.asarray(inputs[n], np.float32) for n in ("Wq", "Wk", "Wv"))
    Wo = np.asarray(inputs["Wo"], np.float32)
    bo = np.asarray(inputs["bo"], np.float32)
    g1, b1 = (np.asarray(inputs["ln1_g"], np.float32),
              np.asarray(inputs["ln1_b"], np.float32))
    g3, b3 = (np.asarray(inputs["ln3_g"], np.float32),
              np.asarray(inputs["ln3_b"], np.float32))
    Wff1 = np.asarray(inputs["Wff1"], np.float32)
    bff1 = np.asarray(inputs["bff1"], np.float32)
    Wff2 = np.asarray(inputs["Wff2"], np.float32)
    bff2 = np.asarray(inputs["bff2"], np.float32)

    Wq_e, Wk_e, Wv_e = g1[:, None] * Wq, g1[:, None] * Wk, g1[:, None] * Wv
    bq_full, bk_full, bv_full = b1 @ Wq, b1 @ Wk, b1 @ Wv
    W1_e = g3[:, None] * Wff1
    b1_full = b3 @ Wff1 + bff1
    b1a_full, b1g_full = b1_full[:FFI], b1_full[FFI:]
    key = (bool(np.any(bq_full)), bool(np.any(bk_full)), bool(np.any(bv_full)),
           bool(np.any(bo)), bool(np.any(b1a_full)), bool(np.any(b1g_full)),
           bool(np.any(bff2)))
    if key not in _CACHE:
        _CACHE[key] = build_program(*key)
    nc = _CACHE[key]

    wo_h = Wo.reshape(KD, 128, D).transpose(1, 0, 2).copy()
    w1_h = W1_e.reshape(KD, 128, 64, 128).transpose(2, 1, 0, 3).copy()
    w2_h = Wff2.reshape(32, 128, D).copy()

    in_maps = []
    for c in range(N_CORES):
        cs = slice(128 * c, 128 * (c + 1))

        def tile_w(W):
            return W[:, cs].reshape(KD, 128, 128).transpose(1, 0, 2).copy()

        in_maps.append({
            "x": x,
            "x_res": x[TPC * c:TPC * (c + 1)].copy(),
            "wq": tile_w(Wq_e), "wk": tile_w(Wk_e), "wv": tile_w(Wv_e),
            "wo": wo_h, "wff1": w1_h, "wff2": w2_h,
            "bq_row": bq_full[None, cs].copy(),
            "bk_row": bk_full[None, cs].copy(),
            "bv_row": bv_full[None, cs].copy(),
            "bo_row": bo[None, :].copy(),
            "b1a": b1a_full[None, :].copy(),
            "b1g": b1g_full.reshape(32, 128, 1).copy(),
            "b2_row": bff2[None, :].copy(),
        })

    res = bass_utils.run_bass_kernel_spmd(
        nc, in_maps, core_ids=list(range(N_CORES)),
        trace=bool(int(os.environ.get("KBENCH_TRACE", "0"))))
    kernel.last_results = res

    xq = np.empty((B * H, N, DH), np.float32)
    xk = np.empty((B * H, N, DH), np.float32)
    xv = np.empty((B * H, N, DH), np.float32)
    io = np.empty((B, H, N, DH), np.float32)
    xfull = np.empty((TOK, D), np.float32)
    for c in range(N_CORES):
        r = res.results[c]
        for pr, (b, hl) in enumerate(PAIRS):
            h = 2 * c + hl
            xq[b * H + h] = r["q_out"][pr]
            xk[b * H + h] = r["k_out"][pr]
            xv[b * H + h] = r["v_out"][pr]
            io[b, h] = r["io_out"][pr]
        xfull[TPC * c:TPC * (c + 1)] = r["x_out"]
    xf = res.results[0]["xf_out"]
    if np.any(g1 != 1.0) or np.any(b1 != 0.0):
        xf = xf * g1 + b1
    return (xfull.reshape(B, N, D), xq, xk, xv, io,
            xf.reshape(B, N, D).astype(np.float32))
